# revision 47
# baseline (speedup 1.0000x reference)
"""Trainium2 Bass kernel for nn_NodeDetector (masked-node GATv2 ensemble).

The reference vmaps a full 2-layer GATv2 over 256 masked-node variants;
variant v differs from the shared base computation in exactly one input
row, so we compute the base graph once and apply sparse incremental
updates per variant (phases: P1 base layer-1 over all 4352 edges, B
per-variant recompute at dst v, A "rare" light pairs, D layer-2 at v).
Per core: 32 variants; shared phases replicated across the 8 cores (a
cross-core collective rendezvous costs ~80us of launch skew under this
runtime, so no collectives).

Implementation notes:
- every gather/scatter is a tensor-engine one-hot matmul; one-hots fp8,
  value tables fp16 (final gate is 2e-2 rel so v1's fp32-exact hi/lo
  pairs are kept only for T1N, whose den/negnum feed a cancellation in
  phase A).
- softmax alphas are invariant to per-dst logit shifts, so the R-side
  attention column (constant per dst) is dropped and a global -K1 shift
  keeps w=exp(logit) and w*u inside fp16 range; exp needs no
  segment-max pass.
- phase 0 builds the GATv2 node tables directly node-major using the
  projection transpose as the stationary operand with an appended
  ones-row carrying biases (no PE transposes), with column-halved
  matmul+act pairs so the ladder pipelines.
- the edge pipeline is engine-skewed 3 deep: gathers/|u|(Scalar)/
  att-mult(GpSimd) at group g, reduce+logit (DVE) at g-1, and
  exp(Scalar)/w*u(DVE)/scatter(PE) at g-2, so every in-order queue has
  a full group of slack per cross-engine dependency; B/D use a
  latency-optimized variant (att-mult on DVE, skew 1).
- serial chains (B-post, A-prep, T1N/g1-base + T2B for the first dst
  half, fp16 conversions) are emitted as callbacks inside the P1
  stream and lean on GpSimd for SBUF-only ops so they ride in pipeline
  slack instead of extending the tail.
"""

import numpy as np
import ml_dtypes

import concourse.bass as bass
import concourse.mybir as mybir
import concourse.tile as tile
from concourse import bacc
from concourse.bass_utils import run_bass_kernel_spmd
from concourse.masks import make_identity

F32 = mybir.dt.float32
FP16 = mybir.dt.float16
BF16 = mybir.dt.bfloat16
FP8 = mybir.dt.float8e4
AF = mybir.ActivationFunctionType
OP = mybir.AluOpType
AX = mybir.AxisListType
FP8NP = ml_dtypes.float8_e4m3

N = 256
NH = 2
NCORES = 8
VPC = 32
ET_P1 = 34
W = 130          # table width: 128 value cols + 2 attention a-cols
NG = 3           # etiles per DVE group


# ------------------------------------------------------------------
# host tables
# ------------------------------------------------------------------

def _build_tables(edge_index):
    src = np.asarray(edge_index[0]).astype(np.int64)
    dst = np.asarray(edge_index[1]).astype(np.int64)
    E = src.shape[0]
    order = np.argsort(dst, kind="stable")
    p1_src, p1_dst = src[order], dst[order]

    p1src8 = np.zeros((128, ET_P1 * 2 * 128), np.float32)
    dst_chunks, sc_halves = [], []
    dst_blocks, sc_blocks = [], []
    for t in range(ET_P1):
        es = slice(128 * t, 128 * (t + 1))
        s_t, d_t = p1_src[es], p1_dst[es]
        for c in range(2):
            m = (s_t // 128) == c
            p1src8[s_t[m] - 128 * c,
                   (2 * t + c) * 128 + np.where(m)[0]] = 1.0
        dl, sl = [], []
        for c in range(2):
            m = (d_t // 128) == c
            if m.any():
                oh = np.zeros((128, 128), np.float32)
                oh[d_t[m] - 128 * c, np.where(m)[0]] = 1.0
                dl.append(c)
                dst_blocks.append(oh)
                sc = np.zeros((128, 128), np.float32)
                sc[np.where(m)[0], d_t[m] - 128 * c] = 1.0
                sl.append(c)
                sc_blocks.append(sc)
        dst_chunks.append(tuple(dl))
        sc_halves.append(tuple(sl))
    p1dst8 = np.concatenate(dst_blocks, axis=1)
    p1sc8 = np.concatenate(sc_blocks, axis=1)

    in_edges_of = [np.where((dst == v) & (src != v))[0] for v in range(N)]
    out_cnt = {}
    for e in range(E):
        if src[e] != dst[e]:
            out_cnt.setdefault(int(src[e]), {})
            d = int(dst[e])
            out_cnt[int(src[e])][d] = out_cnt[int(src[e])].get(d, 0) + 1
    m_self = np.array([((src == v) & (dst == v)).sum() for v in range(N)],
                      np.float32)

    pre = []
    EBs = []
    for c in range(NCORES):
        V = list(range(VPC * c, VPC * (c + 1)))
        el = np.concatenate([in_edges_of[v] for v in V])
        el = el[np.argsort(dst[el], kind="stable")]
        in_set = [set(src[in_edges_of[v]].tolist()) for v in V]
        rare = []
        for vi, v in enumerate(V):
            for d in sorted(out_cnt.get(v, {})):
                if d in in_set[vi]:
                    rare.append((vi, d, out_cnt[v][d]))
        assert len(rare) <= 128, f"rare overflow {len(rare)}"
        EBs.append(-(-len(el) // 128))
        pre.append((V, el, rare))
    EB = max(EBs)

    percore = []
    for c in range(NCORES):
        V, el, rare = pre[c]
        nE = len(el)
        b_src = np.zeros((128, EB * 2 * 128), np.float32)
        d_src = np.zeros((128, EB * 3 * 128), np.float32)
        xr_oh = np.zeros((32, EB * 128), np.float32)
        sc_oh = np.zeros((128, EB * 32), np.float32)
        rare_pos = {(vi, d): i for i, (vi, d, _) in enumerate(rare)}
        for t in range(EB):
            for i in range(128):
                k = 128 * t + i
                if k >= nE:
                    continue
                e = el[k]
                s, v = int(src[e]), int(dst[e])
                vi = v - 32 * c
                ch = s // 128
                b_src[s - 128 * ch, (2 * t + ch) * 128 + i] = 1.0
                if (vi, s) in rare_pos:
                    d_src[rare_pos[(vi, s)], (3 * t + 2) * 128 + i] = 1.0
                else:
                    d_src[s - 128 * ch, (3 * t + ch) * 128 + i] = 1.0
                xr_oh[vi, 128 * t + i] = 1.0
                sc_oh[i, 32 * t + vi] = 1.0
        sv = np.zeros((128, 64), np.float32)
        for vi, v in enumerate(V):
            sv[v % 128, 32 * (v // 128) + vi] = 1.0
        a_d = np.zeros((128, 256), np.float32)
        a_xls = np.zeros((32, 128), np.float32)
        a_xl = np.zeros((128, 256), np.float32)
        a_C = np.zeros((128, 1), np.float32)
        for i, (vi, d, cnt) in enumerate(rare):
            a_d[d % 128, 128 * (d // 128) + i] = 1.0
            a_xls[vi, i] = 1.0
            v = V[vi]
            a_xl[v % 128, 128 * (v // 128) + i] = 1.0
            a_C[i, 0] = cnt
        percore.append({
            "bsrc8": b_src, "dsrc8": d_src, "xr8": xr_oh, "bsc8": sc_oh,
            "sv8": sv, "selfdiag8": np.diag(m_self[V]).astype(np.float32),
            "a_d8": a_d, "a_xls8": a_xls, "a_xl8": a_xl, "a_C": a_C,
        })

    shared = {"p1src8": p1src8, "p1dst8": p1dst8, "p1sc8": p1sc8}
    dims = dict(EB=EB, dst_chunks=tuple(dst_chunks),
                sc_halves=tuple(sc_halves), n_dst=p1dst8.shape[1] // 128,
                n_sc=p1sc8.shape[1] // 128)
    return shared, percore, dims


def _prep_weights(inp):
    f32 = np.float32
    w = {k: np.asarray(v, f32) for k, v in inp.items() if k != "edge_index"}
    att1, att2 = w["g1_att"], w["g2_att"]

    def acol(wmat, att):
        return np.stack([wmat[:, 64 * h:64 * (h + 1)] @ att[h]
                         for h in range(NH)], axis=1).astype(f32)

    def rep(v):
        v = np.asarray(v, f32).reshape(1, -1)
        return np.ascontiguousarray(np.broadcast_to(v, (128, v.shape[1])))

    # Softmax alphas are invariant to any per-dst logit shift, so the
    # R-side acol contribution (constant per dst) is dropped entirely and
    # a global shift -K1 keeps w=exp(logit) and w*u inside fp16 range.
    K1 = 2.5

    def tab65(wmat, b, att, r_shift=None):
        t = np.zeros((65, W), f32)
        t[0:64, 0:128] = wmat
        t[64, 0:128] = b
        if r_shift is None:
            t[0:64, 128:130] = acol(wmat, att)
            t[64, 128:130] = [b[64 * h:64 * (h + 1)] @ att[h]
                              for h in range(NH)]
        else:
            t[64, 128:130] = r_shift
        return t

    blr = w["g2_bl"] + w["g2_br"]
    P = {
        "xT": w["x"].T.copy(), "eT": w["E_emb"].T.copy(),
        "wl_tab": tab65(w["g1_wl"], w["g1_bl"], att1),
        "wr_tab": tab65(w["g1_wr"], w["g1_br"], att1, r_shift=-K1 / 0.6),
        "W2LA": np.concatenate([w["g2_wl"], acol(w["g2_wl"], att2)], axis=1),
        "W2RA": np.concatenate([w["g2_wr"],
                                np.zeros((64, 2), f32)], axis=1),
        "blra": rep(np.concatenate([blr, np.zeros(2, f32)])),
        "att1r": rep(np.concatenate([att1[0], att1[1]]) * 0.4),
        "att2r": rep(np.concatenate([att2[0], att2[1]]) * 0.4),
        "g1bias": rep(w["g1_bias"]),
        "g2bias": rep(w["g2_bias"]),
        "conv_b": w["conv_b"].reshape(128, 1),
        "W_LN": (w["lin2_w"] @ w["normal_proj"]).astype(f32),
        "W_LM": (w["lin2_w"] @ w["masked_proj"]).astype(f32),
        "b_LN": (w["lin2_b"] @ w["normal_proj"]).reshape(64, 1).astype(f32),
        "b_LM": (w["lin2_b"] @ w["masked_proj"]).reshape(64, 1).astype(f32),
        "rec_b": w["rec_b"].reshape(64, 1),
    }
    for nm in ("node_proj", "emb_proj", "conv_w0", "conv_w1", "rec_w"):
        P[nm] = w[nm]
    return P


# ------------------------------------------------------------------
# input packing
# ------------------------------------------------------------------

def _pack_specs(dims):
    EB, n_dst, n_sc = dims["EB"], dims["n_dst"], dims["n_sc"]
    pkh = [("xT", 64, 256), ("eT", 64, 256), ("node_proj", 64, 128),
           ("emb_proj", 64, 128), ("conv_w0", 128, 128),
           ("conv_w1", 128, 128), ("W_LN", 128, 64), ("W_LM", 128, 64),
           ("wl_tab", 65, W), ("wr_tab", 65, W), ("W2LA", 64, W),
           ("W2RA", 64, W), ("rec_w", 64, 64)]
    pkw = [("conv_b", 128, 1), ("b_LN", 64, 1), ("b_LM", 64, 1),
           ("att1r", 128, 128), ("att2r", 128, 128), ("g1bias", 128, 64),
           ("g2bias", 128, 64), ("blra", 128, W), ("a_C", 128, 1),
           ("rec_b", 64, 1)]
    pk8 = [("sv8", 128, 64), ("selfdiag8", 32, 32), ("xr8", 32, EB * 128),
           ("bsrc8", 128, EB * 2 * 128), ("bsc8", 128, EB * 32),
           ("a_xls8", 32, 128), ("a_xl8", 128, 256), ("a_d8", 128, 256),
           ("p1src8", 128, ET_P1 * 2 * 128), ("p1dst8", 128, n_dst * 128),
           ("p1sc8", 128, n_sc * 128), ("dsrc8", 128, EB * 3 * 128)]
    return pkh, pkw, pk8


def _pack_offsets(spec):
    off, pos = {}, 0
    for name, rows, cols in spec:
        off[name] = (rows, pos, cols)
        pos += cols
    return off, pos


def _pack_arrays(spec, src_dict, np_dtype):
    off, total = _pack_offsets(spec)
    arr = np.zeros((128, total), np_dtype)
    for name, rows, cols in spec:
        v = np.asarray(src_dict[name], np.float32)
        assert v.shape == (rows, cols), (name, v.shape, rows, cols)
        arr[0:rows, off[name][1]:off[name][1] + cols] = v.astype(np_dtype)
    return arr


# ------------------------------------------------------------------
# device program
# ------------------------------------------------------------------

def _build_program(dims, dbg=False):
    nc = bacc.Bacc("TRN2", target_bir_lowering=False, debug=False)
    pkh, pkw, pk8 = _pack_specs(dims)
    D = {"_dbg": dbg}
    D["PKH"] = nc.dram_tensor("PKH", [128, _pack_offsets(pkh)[1]], FP16,
                              kind="ExternalInput")
    D["PKW"] = nc.dram_tensor("PKW", [128, _pack_offsets(pkw)[1]], F32,
                              kind="ExternalInput")
    D["PK8"] = nc.dram_tensor("PK8", [128, _pack_offsets(pk8)[1]], FP8,
                              kind="ExternalInput")
    D["outT"] = nc.dram_tensor("outT", [64, VPC], F32, kind="ExternalOutput")
    with tile.TileContext(nc) as tc:
        _trace(nc, tc, D, dims)
    nc.compile()
    return nc


def _trace(nc, tc, D, dims):
    import contextlib
    EB = dims["EB"]
    dst_chunks = dims["dst_chunks"]
    sc_halves = dims["sc_halves"]
    pkh_spec, pkw_spec, pk8_spec = _pack_specs(dims)
    pkh_off = _pack_offsets(pkh_spec)[0]
    pkw_off = _pack_offsets(pkw_spec)[0]
    pk8_off = _pack_offsets(pk8_spec)[0]

    ctx = contextlib.ExitStack()
    with ctx:
        consts = ctx.enter_context(tc.tile_pool(name="consts", bufs=1))
        tabs = ctx.enter_context(tc.tile_pool(name="tabs", bufs=1))
        work = ctx.enter_context(tc.tile_pool(name="work", bufs=4))
        psacc = ctx.enter_context(tc.tile_pool(name="psacc", bufs=1,
                                               space="PSUM"))
        psum = ctx.enter_context(tc.tile_pool(name="psum", bufs=2,
                                              space="PSUM"))

        dma = nc.sync.dma_start
        tt = nc.vector.tensor_tensor
        stt = nc.vector.scalar_tensor_tensor
        red = nc.vector.tensor_reduce
        act = nc.scalar.activation
        mm = nc.tensor.matmul

        import contextlib as _ctxlib

        def scope(name):
            return nc.named_scope(name)

        def dbg_dump(name, ap):
            if not D.get("_dbg"):
                return
            t_ = nc.dram_tensor("dbg_" + name, list(ap.shape), F32,
                                kind="ExternalOutput")
            dma(out=t_[:], in_=ap)

        ident = consts.tile([128, 128], F32, tag="ident")
        make_identity(nc, ident[:])

        # ---- packed input loads (split at consumption boundaries) ----
        PKHt = consts.tile([128, D["PKH"].shape[1]], FP16, tag="PKH")
        h1 = pkh_off["conv_w0"][1]
        h2 = pkh_off["W_LN"][1]
        hsplit = pkh_off["wl_tab"][1]
        dma(out=PKHt[:, 0:h1], in_=D["PKH"][:, 0:h1])
        dma(out=PKHt[:, h1:h2], in_=D["PKH"][:, h1:h2])
        dma(out=PKHt[:, h2:hsplit], in_=D["PKH"][:, h2:hsplit])
        dma(out=PKHt[:, hsplit:], in_=D["PKH"][:, hsplit:])
        PKWt = consts.tile([128, D["PKW"].shape[1]], F32, tag="PKW")
        dma(out=PKWt[:], in_=D["PKW"][:])
        PK8t = consts.tile([128, D["PK8"].shape[1]], FP8, tag="PK8")
        s1 = pk8_off["a_xls8"][1]
        s2 = pk8_off["p1src8"][1]
        s3 = pk8_off["p1dst8"][1]
        s4 = pk8_off["dsrc8"][1]
        dma(out=PK8t[:, 0:s1], in_=D["PK8"][:, 0:s1])
        dma(out=PK8t[:, s1:s2], in_=D["PK8"][:, s1:s2])
        dma(out=PK8t[:, s2:s3], in_=D["PK8"][:, s2:s3])
        dma(out=PK8t[:, s3:s4], in_=D["PK8"][:, s3:s4])
        dma(out=PK8t[:, s4:], in_=D["PK8"][:, s4:])

        def gh(name):
            r, c0, c = pkh_off[name]
            return PKHt[0:r, c0:c0 + c]

        def gw(name):
            r, c0, c = pkw_off[name]
            return PKWt[0:r, c0:c0 + c]

        def g8(name):
            r, c0, c = pk8_off[name]
            return PK8t[0:r, c0:c0 + c]

        att1r = gw("att1r")
        att2r = gw("att2r")
        att1h = consts.tile([128, 128], FP16, tag="att1h")
        nc.vector.tensor_copy(out=att1h[:], in_=att1r)
        att2h = consts.tile([128, 128], FP16, tag="att2h")
        nc.vector.tensor_copy(out=att2h[:], in_=att2r)
        g1bias = gw("g1bias")
        g2bias = gw("g2bias")
        w2la = gh("W2LA")
        w2ra = gh("W2RA")

        def ts_mul(out, in0, s):
            nc.vector.tensor_scalar_mul(out=out, in0=in0, scalar1=s)

        def mk_hilo(src32, blocks, tag):
            # SBUF-only: runs on gpsimd to keep DVE free
            hi = tabs.tile([128, blocks * W], FP16, tag=tag + "_hi")
            nc.gpsimd.tensor_copy(out=hi[:], in_=src32)
            lo32 = work.tile([128, blocks * W], F32, tag=tag + "_lo32")
            nc.gpsimd.tensor_tensor(out=lo32[:], in0=src32, in1=hi[:],
                                    op=OP.subtract)
            lo = tabs.tile([128, blocks * W], BF16, tag=tag + "_lo")
            nc.gpsimd.tensor_copy(out=lo[:], in_=lo32[:])
            return hi, lo

        def elu(x_ap, R, tag, gp=False):
            # elu(x) = relu(x) + min(exp(x), 1) - 1
            ve = nc.gpsimd if gp else nc.vector
            ex = work.tile([R, 64], F32, tag=tag + "_ex")
            rx = work.tile([R, 64], F32, tag=tag + "_rx")
            act(out=ex[:], in_=x_ap, func=AF.Exp)
            act(out=rx[:], in_=x_ap, func=AF.Relu)
            ve.tensor_scalar(out=x_ap, in0=ex[:], scalar1=1.0,
                             scalar2=-1.0, op0=OP.min, op1=OP.add)
            ve.tensor_tensor(out=x_ap, in0=x_ap, in1=rx[:], op=OP.add)

        # ---------------- phase 0 ----------------
        def mmh(lhsT, rhs, M, Nf, tag, bias=None, func=AF.Identity,
                extra=None):
            # column-halved so each act starts as soon as its half's
            # matmul drains (pipelines the phase-0 ladder)
            out_t = tabs.tile([M, Nf], FP16, tag=tag)
            hw = Nf // 2
            for hh in range(2):
                cs = slice(hw * hh, hw * (hh + 1))
                ps = psum.tile([128, 256], F32, tag="ps")
                mm(ps[:M, 0:hw], lhsT, rhs[:, cs], start=True,
                   stop=extra is None)
                if extra is not None:
                    mm(ps[:M, 0:hw], extra[0], extra[1][:, cs],
                       start=False, stop=True)
                if bias is None:
                    act(out=out_t[:, cs], in_=ps[:M, 0:hw], func=func)
                else:
                    act(out=out_t[:, cs], in_=ps[:M, 0:hw], func=func,
                        bias=bias)
            return out_t

        sc_ph0 = nc.enter_named_scope("ph0", True)
        xpT = mmh(gh("node_proj"), gh("xT"), 128, 256, "xpT")
        epT = mmh(gh("emb_proj"), gh("eT"), 128, 256, "epT")
        HbT = mmh(gh("conv_w0"), epT[:], 128, 256, "HbT",
                  bias=gw("conv_b"), func=AF.Tanh,
                  extra=(gh("conv_w1"), xpT[:]))
        HsT = mmh(gh("conv_w0"), epT[:], 128, 256, "HsT",
                  bias=gw("conv_b"), func=AF.Tanh)

        def ptab(src, bias, tag):
            out_t = tabs.tile([65, 256], FP16, tag=tag)
            for hh in range(2):
                cs = slice(128 * hh, 128 * (hh + 1))
                ps = psum.tile([128, 256], F32, tag="ps")
                mm(ps[:64, 0:128], gh("W_LN") if tag == "PbT"
                   else gh("W_LM"), src[:, cs], start=True, stop=True)
                act(out=out_t[0:64, cs], in_=ps[:64, 0:128],
                    func=AF.Identity, bias=bias)
            nc.vector.memset(out_t[64:65, :], 1.0)
            return out_t

        PbT = ptab(HbT[:], gw("b_LN"), "PbT")
        PsT = ptab(HsT[:], gw("b_LM"), "PsT")

        # VT blocks, node-major: [node, 128 vals + 2 acols]
        VTB = tabs.tile([128, 4 * W], FP16, tag="VTB")   # XL c0,c1 XR c0,c1
        VTs = tabs.tile([128, 4 * W], FP16, tag="VTs")   # XLs c0,c1 XRs c0,c1
        VTXR32 = tabs.tile([128, 2 * 128], F32, tag="VTXR32")
        OFF = {"XL": 0, "XR": 2 * W}

        def vt_block(dst_t, col0, PT, wtab, xr32_ch=None):
            ps = psum.tile([128, 256], F32, tag="ps")
            ch = None if xr32_ch is None else xr32_ch
            mm(ps[:, :W], PT, wtab, start=True, stop=True)
            nc.vector.tensor_copy(out=dst_t[:, col0:col0 + W],
                                  in_=ps[:, 0:W])
            if ch is not None:
                act(out=VTXR32[:, 128 * ch:128 * (ch + 1)],
                    in_=ps[:, 0:128], func=AF.Copy)

        for ch in range(2):
            vt_block(VTB, OFF["XL"] + W * ch,
                     PbT[0:65, 128 * ch:128 * (ch + 1)], gh("wl_tab"))
        for ch in range(2):
            vt_block(VTs, W * ch,
                     PsT[0:65, 128 * ch:128 * (ch + 1)], gh("wl_tab"))
        for ch in range(2):
            vt_block(VTs, 2 * W + W * ch,
                     PsT[0:65, 128 * ch:128 * (ch + 1)], gh("wr_tab"))
        for ch in range(2):
            vt_block(VTB, OFF["XR"] + W * ch,
                     PbT[0:65, 128 * ch:128 * (ch + 1)], gh("wr_tab"),
                     xr32_ch=ch)

        def vslice(key, ch):
            return VTB[:, OFF[key] + W * ch:OFF[key] + W * (ch + 1)]

        if D.get("_dbg"):
            vt32 = work.tile([128, 4 * W], F32, tag="dbg_vt32")
            nc.vector.tensor_copy(out=vt32[:], in_=VTB[:])
            dbg_dump("VTB32", vt32[:])

        # ---------------- minis ----------------
        def extract_mini(scol, tag):
            ps = psum.tile([32, W], F32, tag="ps")
            for ch in range(2):
                mm(ps[:], g8("sv8")[:, 32 * ch:32 * (ch + 1)],
                   VTs[:, scol + W * ch:scol + W * (ch + 1)],
                   start=(ch == 0), stop=(ch == 1), skip_group_check=True)
            m32 = tabs.tile([32, W], F32, tag=tag + "32")
            act(out=m32[:], in_=ps[:], func=AF.Copy)
            mh = tabs.tile([32, W], FP16, tag=tag + "h")
            nc.vector.tensor_copy(out=mh[:], in_=ps[:])
            return m32, mh

        nc.leave_named_scope("ph0", sc_ph0[0], True)
        sc_mini = nc.enter_named_scope("mini", True)
        XLsm32, XLsm_h = extract_mini(0, "XLsm")
        XRsm32, XRsm_h = extract_mini(2 * W, "XRsm")
        dbg_dump("XLsm", XLsm32[:])
        dbg_dump("XRsm", XRsm32[:])

        # ---------------- edge machinery ----------------
        def gather_seq(sl, pairs, start=True, stop=True):
            seq = []
            for p in pairs:
                lhs = p[0]
                for rhs_ap in p[1:]:
                    if rhs_ap is not None:
                        seq.append((lhs, rhs_ap))
            n = len(seq)
            for j, (lhs, rhs_ap) in enumerate(seq):
                mm(sl, lhs, rhs_ap, start=(j == 0 and start),
                   stop=(j == n - 1 and stop), skip_group_check=True)

        def run_groups(tag, n_et, gather_emit, scatter_emit, att_h,
                       post_cbs=None, lat=False, pre_slots=None):
            # Batched group stages (DVE op overhead ~100ns makes batched
            # ops strictly cheaper) + a 4-deep PSUM ring so gathers run
            # 3 groups ahead of the DVE/ACT edge stage and the tensor
            # stream never drains (keeps PE HAM-warm).
            groups = [(g0, min(NG, n_et - g0))
                      for g0 in range(0, n_et, NG)]
            st = {}

            def e_gather(gi):
                g0, ng = groups[gi]
                if pre_slots is not None:
                    ps_u = pre_slots[gi]
                else:
                    ps_u = psum.tile([128, NG * W], F32, tag="psu",
                                     bufs=4)
                for i in range(ng):
                    gather_emit(g0 + i, ps_u[:, W * i:W * (i + 1)])
                st[gi] = {"ps": ps_u}

            def e_abs(gi):
                g0, ng = groups[gi]
                s = st[gi]
                psv = s["ps"][:].rearrange("p (i c) -> p i c", i=NG)
                absu = work.tile([128, NG, 128], FP16, tag=tag + "_absu",
                                 bufs=3)
                act(out=absu[:, :ng, :], in_=psv[:, :ng, 0:128],
                    func=AF.Abs)
                s["absu"] = absu

            def e_gpmult(gi):
                _, ng = groups[gi]
                absu = st[gi]["absu"]
                eng = nc.vector if lat else nc.gpsimd
                eng.tensor_tensor(
                    out=absu[:, :ng, :], in0=absu[:, :ng, :],
                    in1=att_h.rearrange("p c -> p () c")
                    .to_broadcast([128, ng, 128]), op=OP.mult)

            def e_redstt(gi):
                g0, ng = groups[gi]
                s = st[gi]
                psv = s["ps"][:].rearrange("p (i c) -> p i c", i=NG)
                absu = s["absu"]
                lgabs = work.tile([128, NG, 2], F32, tag=tag + "_lgabs",
                                  bufs=3)
                red(out=lgabs[:, :ng, :].rearrange("p i h -> p (i h)"),
                    in_=absu[:, :ng, :]
                    .rearrange("p i (h f) -> p (i h) f", h=2),
                    axis=AX.X, op=OP.add)
                w32 = work.tile([128, NG, 2], F32, tag=tag + "_w32",
                                bufs=3)
                stt(out=w32[:, :ng, :], in0=psv[:, :ng, 128:130],
                    scalar=0.6, in1=lgabs[:, :ng, :],
                    op0=OP.mult, op1=OP.add)
                s["w32"] = w32

            def e_exp(gi):
                _, ng = groups[gi]
                w32 = st[gi]["w32"]
                act(out=w32[:, :ng, :], in_=w32[:, :ng, :], func=AF.Exp)

            def e_vlate(gi):
                _, ng = groups[gi]
                s = st[gi]
                psv = s["ps"][:].rearrange("p (i c) -> p i c", i=NG)
                w32 = s["w32"]
                rhs = work.tile([128, NG, W], FP16, tag=tag + "_rhs",
                                bufs=3)
                act(out=rhs[:, :ng, 128:130], in_=w32[:, :ng, :],
                    func=AF.Copy)
                tt(out=rhs[:, :ng, 0:128]
                   .rearrange("p i (h f) -> p i h f", h=2),
                   in0=psv[:, :ng, 0:128]
                   .rearrange("p i (h f) -> p i h f", h=2),
                   in1=w32[:, :ng, :].rearrange("p i h -> p i h ()")
                   .to_broadcast([128, ng, 2, 64]), op=OP.mult)
                s["rhs"] = rhs

            def e_scatter(gi):
                g0, ng = groups[gi]
                rhs = st[gi]["rhs"]
                for i in range(ng):
                    scatter_emit(g0 + i, rhs[:, i, :])

            def e_post(gi):
                g0, ng = groups[gi]
                if post_cbs is not None:
                    for i in range(ng):
                        for cb in post_cbs.get(g0 + i, []):
                            cb()
                st.pop(gi)

            n = len(groups)
            if lat:
                for gi in range(n + 1):
                    if gi < n:
                        e_gather(gi)
                        e_abs(gi)
                    if gi >= 1:
                        e_exp(gi - 1)
                        e_vlate(gi - 1)
                        e_scatter(gi - 1)
                    if gi < n:
                        e_gpmult(gi)
                        e_redstt(gi)
                    if gi >= 1:
                        e_post(gi - 1)
                return
            for gi in range(n + 2):
                if gi < n:
                    e_gather(gi)
                    e_abs(gi)
                    e_gpmult(gi)
                if 1 <= gi <= n:
                    e_redstt(gi - 1)
                if gi >= 2:
                    e_exp(gi - 2)
                    e_vlate(gi - 2)
                    e_scatter(gi - 2)
                    e_post(gi - 2)

        def edge_stage_small(u_sb, R, att_rep, tag):
            absu = work.tile([R, 128], F32, tag=tag + "_absu")
            act(out=absu[:], in_=u_sb[:R, 0:128], func=AF.Abs)
            tt(out=absu[:], in0=absu[:], in1=att_rep[:R, :], op=OP.mult)
            lgabs = work.tile([R, 2], F32, tag=tag + "_lgabs")
            red(out=lgabs[:], in_=absu[:].rearrange("p (h f) -> p h f", h=2),
                axis=AX.X, op=OP.add)
            w32 = work.tile([R, 2], F32, tag=tag + "_w32")
            stt(out=w32[:], in0=u_sb[:R, 128:130], scalar=0.6,
                in1=lgabs[:], op0=OP.mult, op1=OP.add)
            act(out=w32[:], in_=w32[:], func=AF.Exp)
            rhs = work.tile([R, W], FP16, tag=tag + "_rhs")
            nc.vector.tensor_copy(out=rhs[:, 128:130], in_=w32[:])
            tt(out=rhs[:, 0:128].rearrange("p (h f) -> p h f", h=2),
               in0=u_sb[:R, 0:128].rearrange("p (h f) -> p h f", h=2),
               in1=w32[:].rearrange("p h -> p h ()")
               .to_broadcast([R, 2, 64]), op=OP.mult)
            return rhs

        def nd_post(ps_acc, xr_sb, bias_rep, R, tag, gp=False):
            ve = nc.gpsimd if gp else nc.vector
            den = work.tile([R, 2], F32, tag=tag + "_den")
            act(out=den[:], in_=ps_acc[:R, 128:130], func=AF.Copy)
            nn = work.tile([R, 128], F32, tag=tag + "_nn")
            for hd in range(NH):
                stt(out=nn[:, 64 * hd:64 * (hd + 1)],
                    in0=xr_sb[:R, 64 * hd:64 * (hd + 1)],
                    scalar=den[:, hd:hd + 1],
                    in1=ps_acc[:R, 64 * hd:64 * (hd + 1)],
                    op0=OP.mult, op1=OP.subtract)
            recm = work.tile([R, 2], F32, tag=tag + "_recm")
            nc.vector.reciprocal(out=recm[:], in_=den[:])
            ts_mul(recm[:], recm[:], -0.5)
            g = tabs.tile([R, 64], F32, tag=tag + "_g")
            r1 = work.tile([R, 64], F32, tag=tag + "_r1")
            act(out=g[:], in_=nn[:, 0:64], func=AF.Copy,
                scale=recm[:, 0:1])
            act(out=r1[:], in_=nn[:, 64:128], func=AF.Copy,
                scale=recm[:, 1:2])
            ve.tensor_tensor(out=g[:], in0=g[:], in1=r1[:], op=OP.add)
            ve.tensor_tensor(out=g[:], in0=g[:], in1=bias_rep[:R, :],
                             op=OP.add)
            elu(g[:], R, tag + "_elu", gp=gp)
            return g

        # ---------------- B ----------------
        ps_bd = psacc.tile([32, 2 * W], F32, tag="ps_bd")
        ps_b = ps_bd[:, 0:W]
        ps_d = ps_bd[:, W:2 * W]
        u_self = tabs.tile([32, W], F32, tag="u_self")
        tt(out=u_self[:], in0=XLsm32[:], in1=XRsm32[:], op=OP.add)
        rhsS = edge_stage_small(u_self, 32, att1r, "bself")
        mm(ps_b, g8("selfdiag8"), rhsS[:], start=True, stop=False,
           skip_group_check=True)

        bsrc8 = g8("bsrc8")
        xr8 = g8("xr8")
        bsc8 = g8("bsc8")

        def b_gather(t, sl):
            pairs = [(bsrc8[:, (2 * t + ch) * 128:(2 * t + ch + 1) * 128],
                      vslice("XL", ch)) for ch in range(2)]
            pairs.append((xr8[:, 128 * t:128 * (t + 1)], XRsm_h[:]))
            gather_seq(sl, pairs)

        def b_scatter(t, rhs_ap):
            mm(ps_b, bsc8[:, 32 * t:32 * (t + 1)], rhs_ap,
               start=False, stop=(t == EB - 1), skip_group_check=True)

        nc.leave_named_scope("mini", sc_mini[0], True)
        sc_B = nc.enter_named_scope("B", True)
        run_groups("B", EB, b_gather, b_scatter, att1h, lat=True)
        nc.leave_named_scope("B", sc_B[0], True)

        # ---------------- A early gathers + prep (pre-B) ---------------
        sc_A1 = nc.enter_named_scope("Aprep", True)
        a_d8 = g8("a_d8")
        R = {}
        ps_ae = psum.tile([128, NG * W], F32, tag="psu", bufs=4)
        gather_seq(ps_ae[:, 0:W],
                   [(a_d8[:, 128 * ch:128 * (ch + 1)], vslice("XR", ch))
                    for ch in range(2)])
        gather_seq(ps_ae[:, W:2 * W], [(g8("a_xls8"), XLsm_h[:])])
        gather_seq(ps_ae[:, 2 * W:3 * W],
                   [(g8("a_xl8")[:, 128 * ch:128 * (ch + 1)],
                     vslice("XL", ch)) for ch in range(2)])
        xr_d = tabs.tile([128, W], F32, tag="a_xrd")
        act(out=xr_d[:], in_=ps_ae[:, 0:W], func=AF.Copy)
        # xls/xl value copies (ring slot dies here; A2 reads these)
        xlx = tabs.tile([128, 2, 128], F32, tag="a_xlx")
        act(out=xlx[:, 0, :], in_=ps_ae[:, W:W + 128], func=AF.Copy)
        act(out=xlx[:, 1, :], in_=ps_ae[:, 2 * W:2 * W + 128],
            func=AF.Copy)
        # u_no [128, 2, W]: row 0 = u_new (xls+xr), row 1 = u_old
        u_no = tabs.tile([128, 2, W], F32, tag="a_uno")
        tt(out=u_no[:, 0, :], in0=ps_ae[:, W:2 * W], in1=xr_d[:],
           op=OP.add)
        tt(out=u_no[:, 1, :], in0=ps_ae[:, 2 * W:3 * W], in1=xr_d[:],
           op=OP.add)
        nc.leave_named_scope("Aprep", sc_A1[0], True)

        def aprep_chain():
            absu_a = work.tile([128, 2, 128], F32, tag="a_absu")
            act(out=absu_a[:], in_=u_no[:, :, 0:128], func=AF.Abs)
            tt(out=absu_a[:], in0=absu_a[:],
               in1=att1r.rearrange("p c -> p () c")
               .to_broadcast([128, 2, 128]), op=OP.mult)
            lgabs_a = work.tile([128, 2, 2], F32, tag="a_lgabs")
            red(out=lgabs_a[:],
                in_=absu_a[:].rearrange("p i (h f) -> p i h f", h=2),
                axis=AX.X, op=OP.add)
            wno = work.tile([128, 2, 2], F32, tag="a_wno")
            stt(out=wno[:], in0=u_no[:, :, 128:130], scalar=0.6,
                in1=lgabs_a[:], op0=OP.mult, op1=OP.add)
            act(out=wno[:], in_=wno[:], func=AF.Exp)
            ts_mul(wno[:], wno[:], gw("a_C"))
            R["wno"] = wno

        def b_post_1():
            if D.get("_dbg"):
                psb32 = work.tile([32, W], F32, tag="dbg_psb")
                act(out=psb32[:], in_=ps_b, func=AF.Copy)
                dbg_dump("psb", psb32[:])
            R["g1self"] = nd_post(ps_bd, XRsm32[:], g1bias, 32, "bpost",
                                  gp=True)
            dbg_dump("g1self", R["g1self"][:])

        def b_post_2():
            g1self = R["g1self"]
            ps = psum.tile([64, 32], F32, tag="ps")
            nc.tensor.transpose(ps[:], g1self[:], ident[:32, :32])
            g1sT = work.tile([64, 32], FP16, tag="g1sT")
            nc.vector.tensor_copy(out=g1sT[:], in_=ps[:])
            ps2 = psum.tile([32, W], F32, tag="ps")
            mm(ps2[:], g1sT[:], w2ra, start=True, stop=True)
            XR2S = tabs.tile([32, W], F32, tag="XR2S")
            tt(out=XR2S[:], in0=ps2[:], in1=gw("blra")[0:32, :], op=OP.add)
            dbg_dump("XR2S", XR2S[:])
            XR2S_h = tabs.tile([32, W], FP16, tag="XR2S_h")
            nc.gpsimd.tensor_copy(out=XR2S_h[:], in_=XR2S[:])
            R["XR2S"] = XR2S
            R["XR2S_h"] = XR2S_h
            R["g1sT"] = g1sT

        def b_post_3():
            ps3 = psum.tile([32, W], F32, tag="ps")
            mm(ps3[:], R["g1sT"][:], w2la, start=True, stop=True)
            u_ds = tabs.tile([32, W], F32, tag="u_ds")
            tt(out=u_ds[:], in0=ps3[:], in1=R["XR2S"][:], op=OP.add)
            rhsS2 = edge_stage_small(u_ds, 32, att2r, "dself")
            mm(ps_d, g8("selfdiag8"), rhsS2[:], start=True, stop=False,
               skip_group_check=True)

        # ---------------- P1 ----------------
        ps_num = psacc.tile([128, 2 * W], F32, tag="ps_num")
        p1src8 = g8("p1src8")
        p1dst8 = g8("p1dst8")
        p1sc8 = g8("p1sc8")
        dst_pos, pos = {}, 0
        for t in range(ET_P1):
            for c in dst_chunks[t]:
                dst_pos[(t, c)] = pos
                pos += 1
        sc_pos, pos = {}, 0
        for t in range(ET_P1):
            for h in sc_halves[t]:
                sc_pos[(t, h)] = pos
                pos += 1
        first_h = {h: min(t for t in range(ET_P1) if h in sc_halves[t])
                   for h in range(2)}
        last_h = {h: max(t for t in range(ET_P1) if h in sc_halves[t])
                  for h in range(2)}

        def p1_gather(t, sl):
            pairs = [(p1src8[:, (2 * t + ch) * 128:(2 * t + ch + 1) * 128],
                      vslice("XL", ch)) for ch in range(2)]
            for c in dst_chunks[t]:
                j = dst_pos[(t, c)]
                pairs.append((p1dst8[:, 128 * j:128 * (j + 1)],
                              vslice("XR", c)))
            gather_seq(sl, pairs)

        def p1_scatter(t, rhs_ap):
            for h in sc_halves[t]:
                j = sc_pos[(t, h)]
                mm(ps_num[:, W * h:W * (h + 1)],
                   p1sc8[:, 128 * j:128 * (j + 1)], rhs_ap,
                   start=(t == first_h[h]), stop=(t == last_h[h]),
                   skip_group_check=True)

        # T1N (negnum|den) per half + g1 base DVE chain, emitted inside
        # the P1 pipeline so they overlap remaining P1 tiles.
        T1N = tabs.tile([128, 2 * W], F32, tag="T1N")
        g1b2 = work.tile([128, 2, 64], F32, tag="g1b_g")
        T1NB = tabs.tile([128, 2 * W], FP16, tag="T1N_hi")
        T1NBlo = tabs.tile([128, 2 * W], BF16, tag="T1N_lo")
        T2BB = tabs.tile([128, 2 * W], FP16, tag="T2BB")

        def t1n_asm(h):
            pna = ps_num[:, W * h:W * (h + 1)]
            act(out=T1N[:, W * h + 128:W * h + 130], in_=pna[:, 128:130],
                func=AF.Copy)
            for hd in range(NH):
                stt(out=T1N[:, W * h + 64 * hd:W * h + 64 * (hd + 1)],
                    in0=VTXR32[:, 128 * h + 64 * hd:128 * h + 64 * (hd + 1)],
                    scalar=T1N[:, W * h + 128 + hd:W * h + 129 + hd],
                    in1=pna[:, 64 * hd:64 * (hd + 1)],
                    op0=OP.mult, op1=OP.subtract)

        def t1n_half(h, gp=False):
            ve = nc.gpsimd if gp else nc.vector
            # g1 base for this half
            recm2 = work.tile([128, 2, 2], F32, tag="g1b_recm", bufs=2)
            nc.vector.reciprocal(out=recm2[:, h, :],
                                 in_=T1N[:, W * h + 128:W * h + 130])
            ts_mul(recm2[:, h, :], recm2[:, h, :], -0.5)
            r12 = work.tile([128, 2, 64], F32, tag="g1b_r1", bufs=2)
            act(out=g1b2[:, h, :], in_=T1N[:, W * h:W * h + 64],
                func=AF.Copy, scale=recm2[:, h, 0:1])
            act(out=r12[:, h, :], in_=T1N[:, W * h + 64:W * h + 128],
                func=AF.Copy, scale=recm2[:, h, 1:2])
            ve.tensor_tensor(out=g1b2[:, h, :], in0=g1b2[:, h, :],
                             in1=r12[:, h, :], op=OP.add)
            ve.tensor_tensor(out=g1b2[:, h, :], in0=g1b2[:, h, :],
                             in1=g1bias[:, :], op=OP.add)
            xp2 = work.tile([128, 2, 64], F32, tag="g1b_xp", bufs=2)
            act(out=xp2[:, h, :], in_=g1b2[:, h, :], func=AF.Relu)
            act(out=g1b2[:, h, :], in_=g1b2[:, h, :], func=AF.Exp)
            ve.tensor_scalar(out=g1b2[:, h, :], in0=g1b2[:, h, :],
                             scalar1=1.0, scalar2=-1.0,
                             op0=OP.min, op1=OP.add)
            ve.tensor_tensor(out=g1b2[:, h, :], in0=g1b2[:, h, :],
                             in1=xp2[:, h, :], op=OP.add)

        def hilo_half(h, gp=False):
            ve = nc.gpsimd if gp else nc.vector
            sl = slice(W * h, W * (h + 1))
            ve.tensor_copy(out=T1NB[:, sl], in_=T1N[:, sl])
            lo32 = work.tile([128, W], F32, tag="T1N_lo32")
            ve.tensor_tensor(out=lo32[:], in0=T1N[:, sl], in1=T1NB[:, sl],
                             op=OP.subtract)
            ve.tensor_copy(out=T1NBlo[:, sl], in_=lo32[:])

        def t2b_half(h):
            psT = psum.tile([64, 128], F32, tag="ps")
            nc.tensor.transpose(psT[:], g1b2[:, h, :], ident[:])
            gT = work.tile([64, 128], FP16, tag="g1bT")
            nc.vector.tensor_copy(out=gT[:], in_=psT[:])
            ps4 = psum.tile([128, W], F32, tag="ps")
            mm(ps4[:], gT[:], w2la, start=True, stop=True)
            nc.vector.tensor_copy(out=T2BB[:, W * h:W * (h + 1)],
                                  in_=ps4[:])

        lh0 = last_h[0]
        cbs = {2: [aprep_chain], 5: [b_post_1], 8: [b_post_2],
               11: [b_post_3],
               lh0: [lambda: t1n_asm(0)],
               lh0 + 2: [lambda: t1n_half(0, gp=True)],
               lh0 + 4: [lambda: hilo_half(0, gp=True)],
               lh0 + 6: [lambda: t2b_half(0)]}
        assert lh0 > 13 and lh0 + 6 <= ET_P1 - 2
        assert last_h[1] == ET_P1 - 1

        sc_P1 = nc.enter_named_scope("P1", True)
        run_groups("P1", ET_P1, p1_gather, p1_scatter, att1h,
                   post_cbs=cbs)
        nc.leave_named_scope("P1", sc_P1[0], True)
        sc_pp = nc.enter_named_scope("postP1", True)
        t1n_asm(1)
        hilo_half(1, gp=True)
        # A's T1N gathers only need the fp16/lo tables, not the g1-base
        # chain -- emit them now so they overlap t1n_half/t2b work.
        ps_aTt = psum.tile([128, NG * W], F32, tag="psu", bufs=4)
        ps_aT = ps_aTt[:, 0:W]
        gather_seq(ps_aT,
                   [(a_d8[:, 128 * ch:128 * (ch + 1)],
                     T1NB[:, W * ch:W * (ch + 1)]) for ch in range(2)] +
                   [(a_d8[:, 128 * ch:128 * (ch + 1)],
                     T1NBlo[:, W * ch:W * (ch + 1)]) for ch in range(2)])
        t1n_half(1)
        t2b_half(1)
        dbg_dump("T1N", T1N[:])

        nc.leave_named_scope("postP1", sc_pp[0], True)
        sc_A2 = nc.enter_named_scope("A2", True)
        # ---------------- A: rest of chain ----------------------------
        wno = R["wno"]
        # d = wo*xl - wn*xls ; nn = d + t1n_negnum ; den = t1n_den + wn - wo
        d_a = work.tile([128, 2, 64], F32, tag="a_d")
        nn_a = work.tile([128, 2, 64], F32, tag="a_nn")
        tt(out=d_a[:],
           in0=xlx[:, 1, :].rearrange("p (h f) -> p h f", h=2),
           in1=wno[:, 1, :].rearrange("p h -> p h ()")
           .to_broadcast([128, 2, 64]), op=OP.mult)
        tt(out=nn_a[:],
           in0=xlx[:, 0, :].rearrange("p (h f) -> p h f", h=2),
           in1=wno[:, 0, :].rearrange("p h -> p h ()")
           .to_broadcast([128, 2, 64]), op=OP.mult)
        tt(out=d_a[:], in0=d_a[:], in1=nn_a[:], op=OP.subtract)
        tt(out=nn_a[:], in0=d_a[:],
           in1=ps_aT[:, 0:128].rearrange("p (h f) -> p h f", h=2),
           op=OP.add)
        den_a = work.tile([128, 2], F32, tag="a_den")
        tt(out=den_a[:], in0=wno[:, 0, :], in1=wno[:, 1, :], op=OP.subtract)
        tt(out=den_a[:], in0=den_a[:], in1=ps_aT[:, 128:130],
           op=OP.add)
        nc.vector.tensor_scalar_max(out=den_a[:], in0=den_a[:],
                                    scalar1=1e-30)
        recm_a = work.tile([128, 2], F32, tag="a_recm")
        nc.vector.reciprocal(out=recm_a[:], in_=den_a[:])
        ts_mul(recm_a[:], recm_a[:], -0.5)
        g1light = tabs.tile([128, 64], F32, tag="g1light")
        r1a = work.tile([128, 64], F32, tag="a_r1")
        act(out=g1light[:], in_=nn_a[:, 0, :], func=AF.Copy,
            scale=recm_a[:, 0:1])
        act(out=r1a[:], in_=nn_a[:, 1, :], func=AF.Copy,
            scale=recm_a[:, 1:2])
        tt(out=g1light[:], in0=g1light[:], in1=r1a[:], op=OP.add)
        tt(out=g1light[:], in0=g1light[:], in1=g1bias[:], op=OP.add)
        elu(g1light[:], 128, "a_elu")
        dbg_dump("g1light", g1light[:])

        psT = psum.tile([64, 128], F32, tag="ps")
        nc.tensor.transpose(psT[:], g1light[:], ident[:])
        gT = work.tile([64, 128], FP16, tag="g1lT")
        nc.vector.tensor_copy(out=gT[:], in_=psT[:])
        ps5 = psum.tile([128, W], F32, tag="ps")
        mm(ps5[:], gT[:], w2la, start=True, stop=True)
        T2rB = tabs.tile([128, W], FP16, tag="T2rB")
        nc.vector.tensor_copy(out=T2rB[:], in_=ps5[:])

        nc.leave_named_scope("A2", sc_A2[0], True)
        sc_D = nc.enter_named_scope("DD", True)
        # ---------------- D ----------------
        dsrc8 = g8("dsrc8")

        def d_gather(t, sl):
            pairs = [(dsrc8[:, (3 * t + ch) * 128:(3 * t + ch + 1) * 128],
                      T2BB[:, W * ch:W * (ch + 1)]) for ch in range(2)]
            pairs.append((dsrc8[:, (3 * t + 2) * 128:(3 * t + 3) * 128],
                          T2rB[:]))
            pairs.append((xr8[:, 128 * t:128 * (t + 1)],
                          R["XR2S_h"][:]))
            gather_seq(sl, pairs)

        def d_scatter(t, rhs_ap):
            mm(ps_d, bsc8[:, 32 * t:32 * (t + 1)], rhs_ap,
               start=False, stop=(t == EB - 1), skip_group_check=True)

        run_groups("DD", EB, d_gather, d_scatter, att2h, lat=True)
        g2 = nd_post(ps_bd[:, W:], R["XR2S"][:], g2bias, 32, "dpost")
        dbg_dump("g2", g2[:])

        nc.leave_named_scope("DD", sc_D[0], True)
        sc_out = nc.enter_named_scope("out", True)
        # ---------------- out (transposed; host untransposes) ---------
        ps6 = psum.tile([64, 32], F32, tag="ps")
        nc.tensor.transpose(ps6[:], g2[:], ident[:32, :32])
        g2T = work.tile([64, 32], FP16, tag="g2T")
        nc.vector.tensor_copy(out=g2T[:], in_=ps6[:])
        pso = psum.tile([64, 32], F32, tag="ps")
        mm(pso[:], gh("rec_w"), g2T[:], start=True, stop=True)
        outT = work.tile([64, 32], F32, tag="outT")
        act(out=outT[:], in_=pso[:], func=AF.Tanh, bias=gw("rec_b"))
        dma(out=D["outT"][:], in_=outT[:])
        nc.leave_named_scope("out", sc_out[0], True)


# ------------------------------------------------------------------
# entry point
# ------------------------------------------------------------------

_CACHE = {}
TRACE = False
LAST_RESULT = None


def kernel(**inputs):
    global LAST_RESULT
    inputs = {k: np.asarray(v) for k, v in inputs.items()}
    shared, percore, dims = _build_tables(inputs["edge_index"])
    P = _prep_weights(inputs)
    key = (dims["EB"], dims["dst_chunks"], dims["sc_halves"])
    if key not in _CACHE:
        _CACHE[key] = _build_program(dims)
    nc = _CACHE[key]
    pkh, pkw, pk8 = _pack_specs(dims)
    in_maps = []
    for c in range(NCORES):
        src = dict(P)
        src.update(shared)
        src.update(percore[c])
        in_maps.append({
            "PKH": _pack_arrays(pkh, src, np.float16),
            "PKW": _pack_arrays(pkw, src, np.float32),
            "PK8": _pack_arrays(pk8, src, FP8NP),
        })
    kw = {}
    if TRACE:
        kw = dict(trace=True, trace_cores=list(range(NCORES)))
    res = run_bass_kernel_spmd(nc, in_maps, core_ids=list(range(NCORES)),
                               **kw)
    LAST_RESULT = res
    out = np.concatenate([res.results[c]["outT"].T for c in range(NCORES)],
                         axis=0)
    return out.astype(np.float32)
